# revision 9
# baseline (speedup 1.0000x reference)
"""Trainium2 Bass kernel for the CGIM sparse-attention block.

Per-sample math (reference):
  Qf = Wq @ [F1;F2] + bq            (1x1 conv, transposed-layout on device)
  Qs = softmax_d(Qf per head)
  per branch i: K = Wk_i @ F_i (+bk_i cancels), V = Wv_i @ F_i + bv_i
                Ks = softmax_hw(K);  Att = Ks @ Qs;  Xw = Att @ V
  fused = concat(mu*X1 + F1, mu*X2 + F2)
  y = relu(BN(conv3x3(fused, Wc)))

Sharding: data-parallel over batch (B=8) across the 8 NeuronCores; weights
replicated. Each core computes one sample end to end; no collectives.

Key device-side algebra:
 - K softmax bias cancels (constant along softmax axis) -> dropped.
 - K softmax denominator S_d is postponed all the way to the Xw epilogue,
   where d is the partition axis (fused scale mu/S_d).
 - Q/K computed directly in transposed [hw, c] layout (stationary = F tile),
   so Q's per-head softmax is a free-dim segmented reduce and the Att
   contraction over hw needs no transposes at all.
 - AttT computed directly as lhsT for the Xw matmul; only the 32x32
   diagonal (per-head) blocks are copied into a zeroed block-diag tile.
 - conv3x3 = 36 accumulated shifted 1x1 matmuls over a zero-padded
   [66 x 72] image layout; BN+ReLU folded into the PSUM->SBUF ACT.
All matmul operands bf16, fp32 PSUM accumulation.
"""

import numpy as np
import ml_dtypes

import concourse.bass as bass
import concourse.mybir as mybir
import concourse.tile as tile
from concourse import bacc
from concourse.bass_utils import run_bass_kernel_spmd

BF16 = mybir.dt.bfloat16
F32 = mybir.dt.float32
AF = mybir.ActivationFunctionType
ALU = mybir.AluOpType
AX = mybir.AxisListType

B, C, H, W = 8, 256, 64, 64
HW = H * W                  # 4096
NH, D = 8, 32               # heads, per-head dim
NT = HW // 128              # 32 hw-tiles of 128
PH, PW = H + 2, 72          # padded conv image (66 rows x 72 cols, >=66 used)
N_CORES = 8
BN_EPS = 1e-5

_bf = ml_dtypes.bfloat16


def _build_program() -> bass.Bass:
    nc = bacc.Bacc("TRN2", target_bir_lowering=False)

    # ---- DRAM I/O (per core) ----
    f1_d = nc.dram_tensor("f1", [C, HW], BF16, kind="ExternalInput").ap()
    f2_d = nc.dram_tensor("f2", [C, HW], BF16, kind="ExternalInput").ap()
    wq_d = nc.dram_tensor("wq", [128, 4, 256], BF16, kind="ExternalInput").ap()
    wk_d = nc.dram_tensor("wk", [128, 2, 2, 256], BF16, kind="ExternalInput").ap()
    wv_d = nc.dram_tensor("wv", [128, 2, 2, 256], BF16, kind="ExternalInput").ap()
    wc_d = nc.dram_tensor("wc", [128, 4, 18, 128], BF16, kind="ExternalInput").ap()
    bq_d = nc.dram_tensor("bq", [1, 256], BF16, kind="ExternalInput").ap()
    bv_d = nc.dram_tensor("bv", [128, 2, 2], F32, kind="ExternalInput").ap()
    bns_d = nc.dram_tensor("bns", [128, 2], F32, kind="ExternalInput").ap()
    bnb_d = nc.dram_tensor("bnb", [128, 2], F32, kind="ExternalInput").ap()
    muv_d = nc.dram_tensor("muv", [128, 1], F32, kind="ExternalInput").ap()
    y_d = nc.dram_tensor("y", [C, HW], F32, kind="ExternalOutput").ap()

    with tile.TileContext(nc) as tc:
        with tc.tile_pool(name="per", bufs=1) as per, \
             tc.tile_pool(name="sml", bufs=4) as sml:

            # ---- persistent SBUF ----
            wq = per.tile([128, 4, 256], BF16)
            nc.sync.dma_start(wq, wq_d)
            wk = per.tile([128, 2, 2, 256], BF16)
            nc.sync.dma_start(wk, wk_d)
            wv = per.tile([128, 2, 2, 256], BF16)
            nc.sync.dma_start(wv, wv_d)
            wc = per.tile([128, 4, 18, 128], BF16)
            nc.sync.dma_start(wc, wc_d)
            bq = per.tile([1, 256], BF16)
            nc.sync.dma_start(bq, bq_d)
            bv = per.tile([128, 2, 2], F32)
            nc.sync.dma_start(bv, bv_d)
            bns = per.tile([128, 2], F32)
            nc.sync.dma_start(bns, bns_d)
            bnb = per.tile([128, 2], F32)
            nc.sync.dma_start(bnb, bnb_d)
            muv = per.tile([128, 1], F32)
            nc.sync.dma_start(muv, muv_d)

            ones_row = per.tile([1, 128], BF16)
            nc.vector.memset(ones_row, 1.0)
            ones_col = per.tile([128, 1], BF16)
            nc.gpsimd.memset(ones_col, 1.0)

            fbf = per.tile([128, 4, HW], BF16)        # [F1;F2] as 4 ci-tiles
            qse = per.tile([128, NT, 256], BF16)      # exp(QfT), then normalized
            kst1 = per.tile([128, NT, 256], BF16)     # exp(K1fT)
            kst2 = per.tile([128, NT, 256], BF16)     # exp(K2fT)
            vsb1 = per.tile([128, 2, HW], BF16)       # V1, 2 m-groups
            vsb2 = per.tile([128, 2, HW], BF16)
            fp = [per.tile([128, PH, PW], BF16, tag=f"fp{j}", name=f"fp{j}")
                  for j in range(4)]

            for j in range(4):
                nc.gpsimd.memset(fp[j], 0.0)

            # ---- load F (bf16 on host; chunk-outer so compute starts early)
            f_src = [f1_d, f1_d, f2_d, f2_d]
            NCHUNK = 4
            csz = HW // NCHUNK
            for ch in range(NCHUNK):
                for ci in range(4):
                    half = (ci % 2) * 128
                    nc.sync.dma_start(
                        fbf[:, ci, ch * csz:(ch + 1) * csz],
                        f_src[ci][half:half + 128, ch * csz:(ch + 1) * csz])

            # ================= Phase 1: transposed Q/K1/K2 + softmax pieces
            with tc.tile_pool(name="pq", bufs=2, space="PSUM") as pq, \
                 tc.tile_pool(name="pk", bufs=2, space="PSUM") as pk, \
                 tc.tile_pool(name="pss", bufs=1, space="PSUM") as pss:

                ps_s1 = pss.tile([1, 256], F32, tag="s1")
                ps_s2 = pss.tile([1, 256], F32, tag="s2")

                LAG = 2
                def emit_ssum(n):
                    nc.tensor.matmul(ps_s1, ones_col, kst1[:, n, :],
                                     start=(n == 0), stop=(n == NT - 1))
                    nc.tensor.matmul(ps_s2, ones_col, kst2[:, n, :],
                                     start=(n == 0), stop=(n == NT - 1))

                for n in range(NT):
                    psq = pq.tile([128, 256], F32, tag="q")
                    psk1 = pk.tile([128, 256], F32, tag="k1")
                    psk2 = pk.tile([128, 256], F32, tag="k2")
                    for ci in range(4):
                        lhsT = fbf[:, ci, n * 128:(n + 1) * 128]
                        nc.tensor.matmul(psq, lhsT, wq[:, ci, :],
                                         start=(ci == 0), stop=False)
                        if ci < 2:
                            nc.tensor.matmul(psk1, lhsT, wk[:, 0, ci, :],
                                             start=(ci == 0), stop=(ci == 1))
                        else:
                            nc.tensor.matmul(psk2, lhsT, wk[:, 1, ci - 2, :],
                                             start=(ci == 2), stop=(ci == 3))
                    nc.tensor.matmul(psq, ones_row, bq, start=False, stop=True)

                    nc.scalar.activation(kst1[:, n, :], psk1, AF.Exp)
                    nc.scalar.activation(kst2[:, n, :], psk2, AF.Exp)
                    nc.scalar.activation(qse[:, n, :], psq, AF.Exp)

                    # per-head softmax denominator + normalize (in place)
                    q3 = qse[:, n, :].rearrange("p (h e) -> p h e", h=NH)
                    rq = sml.tile([128, NH], F32, tag="rq")
                    nc.vector.tensor_reduce(rq, q3, axis=AX.X, op=ALU.add)
                    rr = sml.tile([128, NH], F32, tag="rr")
                    nc.vector.reciprocal(rr, rq)
                    nc.vector.tensor_mul(q3, q3, rr.to_broadcast([128, NH, D]))

                    if n >= LAG:
                        emit_ssum(n - LAG)
                for n in range(NT - LAG, NT):
                    emit_ssum(n)

                # 1/S rows -> per-partition columns (tiny SBUF->SBUF DMAs)
                scale = {}
                for br, ps_s in ((0, ps_s1), (1, ps_s2)):
                    rs = sml.tile([1, 256], F32, tag="rs")
                    nc.vector.reciprocal(rs, ps_s)
                    for m in range(2):
                        col = sml.tile([128, 1], F32, tag="scat")
                        nc.sync.dma_start(col, rs[0:1, m * 128:(m + 1) * 128])
                        sc = sml.tile([128, 1], F32, tag="scale")
                        nc.vector.tensor_mul(sc, col, muv)   # mu / S_d
                        scale[(br, m)] = sc

            # ================= Phase 2a: V convs (normal layout)
            with tc.tile_pool(name="pv", bufs=8, space="PSUM") as pv:
                for br, (vsb, fci0) in enumerate(((vsb1, 0), (vsb2, 2))):
                    for m in range(2):
                        pst = [pv.tile([128, 512], F32, tag="v", name=f"psv{br}{m}{i}")
                               for i in range(8)]
                        for ci in range(2):
                            lhsT = wv[:, br, ci, m * 128:(m + 1) * 128]
                            for n8 in range(8):
                                nc.tensor.matmul(
                                    pst[n8], lhsT,
                                    fbf[:, fci0 + ci, n8 * 512:(n8 + 1) * 512],
                                    start=(ci == 0), stop=(ci == 1))
                        for n8 in range(8):
                            nc.scalar.activation(
                                vsb[:, m, n8 * 512:(n8 + 1) * 512], pst[n8],
                                AF.Identity, bias=bv[:, br, m:m + 1])

            # ================= Phase 2b: AttT (cross-head), block-diag, Xw
            with tc.tile_pool(name="pa", bufs=4, space="PSUM") as pa, \
                 tc.tile_pool(name="px", bufs=4, space="PSUM") as px:
                psa = {}
                for g in range(2):
                    gs = slice(g * 128, (g + 1) * 128)
                    for br, kst in ((0, kst1), (1, kst2)):
                        p = pa.tile([128, 128], F32, tag="a")
                        psa[(br, g)] = p
                        for n in range(NT):
                            nc.tensor.matmul(p, qse[:, n, gs], kst[:, n, gs],
                                             start=(n == 0), stop=(n == NT - 1))

                attbd = {}
                for (br, g), p in psa.items():
                    t = sml.tile([128, 128], BF16, tag="attbd")
                    nc.vector.memset(t, 0.0)
                    for hb in range(4):
                        hs = slice(hb * 32, (hb + 1) * 32)
                        nc.any.tensor_copy(t[hs, hs], p[hs, hs])
                    attbd[(br, g)] = t

                for br, (vsb, fci0) in enumerate(((vsb1, 0), (vsb2, 2))):
                    for g in range(2):
                        for n8 in range(8):
                            pxt = px.tile([128, 512], F32, tag="x")
                            nc.tensor.matmul(
                                pxt, attbd[(br, g)],
                                vsb[:, g, n8 * 512:(n8 + 1) * 512],
                                start=True, stop=True)
                            # fused = (Xw_raw * mu/S_d) + F  -> padded layout
                            j = 2 * br + g
                            out = fp[j][:, 1 + n8 * 8:9 + n8 * 8, 1:65]
                            nc.vector.scalar_tensor_tensor(
                                out=out,
                                in0=pxt, scalar=scale[(br, g)],
                                in1=fbf[:, fci0 + g, n8 * 512:(n8 + 1) * 512],
                                op0=ALU.mult, op1=ALU.add)

            # ================= Phase 3: conv3x3 + BN + ReLU
            with tc.tile_pool(name="pc", bufs=8, space="PSUM") as pc:
                for m in range(2):
                    pst = [pc.tile([128, 512], F32, tag="c", name=f"psc{m}{i}")
                           for i in range(8)]
                    first, last = (0, 0, 0), (3, 2, 2)
                    for ci in range(4):
                        for dy in range(3):
                            for dx in range(3):
                                lhsT = wc[:, ci, (dy * 3 + dx) * 2 + m, :]
                                for n8 in range(8):
                                    rhs = fp[ci][:, n8 * 8 + dy:n8 * 8 + dy + 8,
                                                 dx:dx + 64]
                                    nc.tensor.matmul(
                                        pst[n8], lhsT, rhs,
                                        start=((ci, dy, dx) == first),
                                        stop=((ci, dy, dx) == last))
                    for n8 in range(8):
                        ysb = sml.tile([128, 512], F32, tag="y")
                        nc.scalar.activation(ysb, pst[n8], AF.Relu,
                                             bias=bnb[:, m:m + 1],
                                             scale=bns[:, m:m + 1])
                        nc.sync.dma_start(
                            y_d[m * 128:(m + 1) * 128,
                                n8 * 512:(n8 + 1) * 512], ysb)
    nc.compile()
    return nc


_PROGRAM = None


def _get_program():
    global _PROGRAM
    if _PROGRAM is None:
        _PROGRAM = _build_program()
    return _PROGRAM


def kernel(F1, F2, Wq, bq, Wk1, bk1, Wv1, bv1, Wk2, bk2, Wv2, bv2,
           mu, Wc, gamma, beta, rmean, rvar):
    f32 = np.float32
    F1 = np.asarray(F1, f32)
    F2 = np.asarray(F2, f32)

    def tile_T(w):   # [O, Cin] -> [128, Cin//128, O] (lhsT tiles)
        wt = np.ascontiguousarray(np.asarray(w, f32).T)      # [Cin, O]
        cin, o = wt.shape
        return wt.reshape(cin // 128, 128, o).transpose(1, 0, 2).astype(_bf)

    wq_h = np.ascontiguousarray(tile_T(Wq))                  # [128, 4, 256]
    wk_h = np.ascontiguousarray(
        np.stack([tile_T(Wk1), tile_T(Wk2)], axis=1))        # [128,2,2,256]
    wv_h = np.ascontiguousarray(
        np.stack([tile_T(Wv1), tile_T(Wv2)], axis=1))

    Wc = np.asarray(Wc, f32)                                 # [256, 512, 3, 3]
    # wc[p, ci, (dy*3+dx)*2+m, col] = Wc[m*128+col, ci*128+p, dy, dx]
    wc_h = Wc.reshape(2, 128, 4, 128, 3, 3)                  # m,col,ci,p,dy,dx
    wc_h = wc_h.transpose(3, 2, 4, 5, 0, 1)                  # p,ci,dy,dx,m,col
    wc_h = np.ascontiguousarray(
        wc_h.reshape(128, 4, 18, 128)).astype(_bf)

    bq_h = np.asarray(bq, f32).reshape(1, 256).astype(_bf)
    # bv_h[p, br, m] = bv_br[m*128 + p]
    bv_h = np.ascontiguousarray(
        np.stack([np.asarray(bv1, f32), np.asarray(bv2, f32)],
                 axis=0).reshape(2, 2, 128).transpose(2, 0, 1))
    inv = np.asarray(gamma, f32) / np.sqrt(np.asarray(rvar, f32) + BN_EPS)
    b2 = np.asarray(beta, f32) - np.asarray(rmean, f32) * inv
    bns_h = np.ascontiguousarray(inv.reshape(2, 128).T)      # [128, 2]
    bnb_h = np.ascontiguousarray(b2.reshape(2, 128).T)
    muv_h = np.full((128, 1), np.asarray(mu, f32).reshape(-1)[0], f32)

    shared = dict(wq=wq_h, wk=wk_h, wv=wv_h, wc=wc_h, bq=bq_h, bv=bv_h,
                  bns=bns_h, bnb=bnb_h, muv=muv_h)
    in_maps = [dict(f1=np.ascontiguousarray(F1[b].reshape(C, HW)).astype(_bf),
                    f2=np.ascontiguousarray(F2[b].reshape(C, HW)).astype(_bf),
                    **shared) for b in range(N_CORES)]

    nc = _get_program()
    res = run_bass_kernel_spmd(nc, in_maps, list(range(N_CORES)))
    kernel.last_results = res

    out = np.stack([res.results[b]["y"] for b in range(N_CORES)])
    return out.reshape(B, C, H, W)


kernel.last_results = None


# revision 10
# speedup vs baseline: 1.1415x; 1.1415x over previous
"""Trainium2 Bass kernel for the CGIM sparse-attention block.

Per-sample math (reference):
  Qf = Wq @ [F1;F2] + bq            (1x1 conv, transposed-layout on device)
  Qs = softmax_d(Qf per head)
  per branch i: K = Wk_i @ F_i (+bk_i cancels), V = Wv_i @ F_i + bv_i
                Ks = softmax_hw(K);  Att = Ks @ Qs;  Xw = Att @ V
  fused = concat(mu*X1 + F1, mu*X2 + F2)
  y = relu(BN(conv3x3(fused, Wc)))

Sharding: data-parallel over batch (B=8) across the 8 NeuronCores; weights
replicated. Each core computes one sample end to end; no collectives.

Key device-side algebra:
 - K softmax bias cancels (constant along softmax axis) -> dropped.
 - K softmax denominator S_d is postponed all the way to the Xw epilogue,
   where d is the partition axis (fused scale mu/S_d).
 - Q/K computed directly in transposed [hw, c] layout (stationary = F tile),
   so Q's per-head softmax is a free-dim segmented reduce and the Att
   contraction over hw needs no transposes at all.
 - AttT computed directly as lhsT for the Xw matmul; only the 32x32
   diagonal (per-head) blocks are copied into a zeroed block-diag tile.
 - conv3x3 = 36 accumulated shifted 1x1 matmuls over a zero-padded
   [66 x 72] image layout; BN+ReLU folded into the PSUM->SBUF ACT.
All matmul operands bf16, fp32 PSUM accumulation.
"""

import numpy as np
import ml_dtypes

import concourse.bass as bass
import concourse.mybir as mybir
import concourse.tile as tile
from concourse import bacc
from concourse.bass_utils import run_bass_kernel_spmd

BF16 = mybir.dt.bfloat16
F32 = mybir.dt.float32
AF = mybir.ActivationFunctionType
ALU = mybir.AluOpType
AX = mybir.AxisListType

B, C, H, W = 8, 256, 64, 64
HW = H * W                  # 4096
NH, D = 8, 32               # heads, per-head dim
NT = HW // 128              # 32 hw-tiles of 128
PH, PW = H + 2, 72          # padded conv image (66 rows x 72 cols, >=66 used)
N_CORES = 8
BN_EPS = 1e-5

_bf = ml_dtypes.bfloat16


def _build_program() -> bass.Bass:
    nc = bacc.Bacc("TRN2", target_bir_lowering=False)

    # ---- DRAM I/O (per core) ----
    f1_d = nc.dram_tensor("f1", [C, HW], BF16, kind="ExternalInput").ap()
    f2_d = nc.dram_tensor("f2", [C, HW], BF16, kind="ExternalInput").ap()
    wq_d = nc.dram_tensor("wq", [128, 4, 256], BF16, kind="ExternalInput").ap()
    wk_d = nc.dram_tensor("wk", [128, 2, 2, 256], BF16, kind="ExternalInput").ap()
    wv_d = nc.dram_tensor("wv", [128, 2, 2, 256], BF16, kind="ExternalInput").ap()
    wc_d = nc.dram_tensor("wc", [128, 4, 18, 128], BF16, kind="ExternalInput").ap()
    bq_d = nc.dram_tensor("bq", [1, 256], BF16, kind="ExternalInput").ap()
    bv_d = nc.dram_tensor("bv", [128, 2, 2], F32, kind="ExternalInput").ap()
    bns_d = nc.dram_tensor("bns", [128, 2], F32, kind="ExternalInput").ap()
    bnb_d = nc.dram_tensor("bnb", [128, 2], F32, kind="ExternalInput").ap()
    muv_d = nc.dram_tensor("muv", [128, 1], F32, kind="ExternalInput").ap()
    y_d = nc.dram_tensor("y", [C, HW], F32, kind="ExternalOutput").ap()

    with tile.TileContext(nc) as tc:
        with tc.tile_pool(name="per", bufs=1) as per, \
             tc.tile_pool(name="sml", bufs=4) as sml:

            # ---- persistent SBUF tiles ----
            wq = per.tile([128, 4, 256], BF16)
            wk = per.tile([128, 2, 2, 256], BF16)
            wv = per.tile([128, 2, 2, 256], BF16)
            wc = per.tile([128, 4, 18, 128], BF16)
            bq = per.tile([1, 256], BF16)
            bv = per.tile([128, 2, 2], F32)
            bns = per.tile([128, 2], F32)
            bnb = per.tile([128, 2], F32)
            muv = per.tile([128, 1], F32)

            ones_row = per.tile([1, 128], BF16)
            nc.vector.memset(ones_row, 1.0)
            ones_col = per.tile([128, 1], BF16)
            nc.gpsimd.memset(ones_col, 1.0)

            fbf = per.tile([128, 4, HW], BF16)        # [F1;F2] as 4 ci-tiles
            qse = per.tile([128, NT, 256], BF16)      # exp(QfT), then normalized
            kst = per.tile([128, NT, 512], BF16)      # [exp(K1fT) | exp(K2fT)]
            vsb1 = per.tile([128, 2, HW], BF16)       # V1, 2 m-groups
            vsb2 = per.tile([128, 2, HW], BF16)
            fp = [per.tile([128, PH, PW], BF16, tag=f"fp{j}", name=f"fp{j}")
                  for j in range(4)]

            # small weights needed first; F loads right behind (DMA priority
            # follows program order). wc/bns/bnb are emitted just before the
            # conv, ~100us later.
            nc.sync.dma_start(wq, wq_d)
            nc.sync.dma_start(wk, wk_d)
            nc.sync.dma_start(bq, bq_d)

            f_src = [f1_d, f1_d, f2_d, f2_d]
            NCHUNK = 4
            csz = HW // NCHUNK
            for ch in range(NCHUNK):
                for ci in range(4):
                    half = (ci % 2) * 128
                    nc.sync.dma_start(
                        fbf[:, ci, ch * csz:(ch + 1) * csz],
                        f_src[ci][half:half + 128, ch * csz:(ch + 1) * csz])

            nc.sync.dma_start(wv, wv_d)
            nc.sync.dma_start(bv, bv_d)
            nc.sync.dma_start(muv, muv_d)
            for j in range(4):
                nc.gpsimd.memset(fp[j], 0.0)

            # ================= Phase 1: transposed Q/K1/K2 + softmax pieces
            with tc.tile_pool(name="pq", bufs=2, space="PSUM") as pq, \
                 tc.tile_pool(name="pk", bufs=2, space="PSUM") as pk, \
                 tc.tile_pool(name="pss", bufs=1, space="PSUM") as pss:

                ps_s = pss.tile([1, 512], F32, tag="s")

                LAG = 2
                def emit_ssum(n):
                    nc.tensor.matmul(ps_s, ones_col, kst[:, n, :],
                                     start=(n == 0), stop=(n == NT - 1))

                for n in range(NT):
                    psq = pq.tile([128, 256], F32, tag="q")
                    psk1 = pk.tile([128, 256], F32, tag="k1")
                    psk2 = pk.tile([128, 256], F32, tag="k2")
                    for ci in range(4):
                        lhsT = fbf[:, ci, n * 128:(n + 1) * 128]
                        nc.tensor.matmul(psq, lhsT, wq[:, ci, :],
                                         start=(ci == 0), stop=False)
                        if ci < 2:
                            nc.tensor.matmul(psk1, lhsT, wk[:, 0, ci, :],
                                             start=(ci == 0), stop=(ci == 1))
                        else:
                            nc.tensor.matmul(psk2, lhsT, wk[:, 1, ci - 2, :],
                                             start=(ci == 2), stop=(ci == 3))
                    nc.tensor.matmul(psq, ones_row, bq, start=False, stop=True)

                    nc.scalar.activation(kst[:, n, 0:256], psk1, AF.Exp)
                    nc.scalar.activation(kst[:, n, 256:512], psk2, AF.Exp)
                    nc.scalar.activation(qse[:, n, :], psq, AF.Exp)

                    # per-head softmax denominator + normalize (in place)
                    q3 = qse[:, n, :].rearrange("p (h e) -> p h e", h=NH)
                    rq = sml.tile([128, NH], F32, tag="rq")
                    nc.vector.tensor_reduce(rq, q3, axis=AX.X, op=ALU.add)
                    rr = sml.tile([128, NH], F32, tag="rr")
                    nc.vector.reciprocal(rr, rq)
                    nc.vector.tensor_mul(q3, q3, rr.to_broadcast([128, NH, D]))

                    if n >= LAG:
                        emit_ssum(n - LAG)
                for n in range(NT - LAG, NT):
                    emit_ssum(n)

                # 1/S row -> per-partition columns (tiny SBUF->SBUF DMAs)
                scale = {}
                rs = sml.tile([1, 512], F32, tag="rs")
                nc.vector.reciprocal(rs, ps_s)
                for br in range(2):
                    for m in range(2):
                        col = sml.tile([128, 1], F32, tag="scat")
                        nc.sync.dma_start(
                            col, rs[0:1, br * 256 + m * 128:br * 256 + (m + 1) * 128])
                        sc = sml.tile([128, 1], F32, tag="scale")
                        nc.vector.tensor_mul(sc, col, muv)   # mu / S_d
                        scale[(br, m)] = sc

            # ================= Phase 2a: AttT (cross-head) + block-diag
            with tc.tile_pool(name="pa", bufs=4, space="PSUM") as pa:
                psa = {}
                for g in range(2):
                    for br in range(2):
                        p = pa.tile([128, 128], F32, tag="a", name=f"psa{br}{g}")
                        psa[(br, g)] = p
                for g in range(2):
                    gs = slice(g * 128, (g + 1) * 128)
                    for n in range(NT):
                        lhsT = qse[:, n, gs]
                        for br in range(2):
                            nc.tensor.matmul(
                                psa[(br, g)], lhsT,
                                kst[:, n, br * 256 + g * 128:br * 256 + (g + 1) * 128],
                                start=(n == 0), stop=(n == NT - 1))

                attbd = {}
                for (br, g), p in psa.items():
                    t = sml.tile([128, 128], BF16, tag="attbd")
                    nc.vector.memset(t, 0.0)
                    for hb in range(4):
                        hs = slice(hb * 32, (hb + 1) * 32)
                        nc.any.tensor_copy(t[hs, hs], p[hs, hs])
                    attbd[(br, g)] = t

            # ================= Phase 2b: V convs, Xw, fused epilogue
            with tc.tile_pool(name="pv", bufs=4, space="PSUM") as pv, \
                 tc.tile_pool(name="px", bufs=4, space="PSUM") as px:
                for br, (vsb, fci0) in enumerate(((vsb1, 0), (vsb2, 2))):
                    for m in range(2):
                        for n8 in range(8):
                            psv = pv.tile([128, 512], F32, tag="v",
                                          name=f"psv{br}{m}{n8}")
                            for ci in range(2):
                                nc.tensor.matmul(
                                    psv, wv[:, br, ci, m * 128:(m + 1) * 128],
                                    fbf[:, fci0 + ci, n8 * 512:(n8 + 1) * 512],
                                    start=(ci == 0), stop=(ci == 1))
                            nc.scalar.activation(
                                vsb[:, m, n8 * 512:(n8 + 1) * 512], psv,
                                AF.Identity, bias=bv[:, br, m:m + 1])

                for br, (vsb, fci0) in enumerate(((vsb1, 0), (vsb2, 2))):
                    for g in range(2):
                        for n8 in range(8):
                            pxt = px.tile([128, 512], F32, tag="x")
                            nc.tensor.matmul(
                                pxt, attbd[(br, g)],
                                vsb[:, g, n8 * 512:(n8 + 1) * 512],
                                start=True, stop=True)
                            # fused = (Xw_raw * mu/S_d) + F  -> padded layout
                            j = 2 * br + g
                            out = fp[j][:, 1 + n8 * 8:9 + n8 * 8, 1:65]
                            nc.vector.scalar_tensor_tensor(
                                out=out,
                                in0=pxt, scalar=scale[(br, g)],
                                in1=fbf[:, fci0 + g, n8 * 512:(n8 + 1) * 512],
                                op0=ALU.mult, op1=ALU.add)

            # ================= Phase 3: conv3x3 + BN + ReLU
            nc.sync.dma_start(wc, wc_d)
            nc.sync.dma_start(bns, bns_d)
            nc.sync.dma_start(bnb, bnb_d)
            with tc.tile_pool(name="pc", bufs=8, space="PSUM") as pc:
                for m in range(2):
                    for hf in range(2):
                        pst = [pc.tile([128, 512], F32, tag="c",
                                       name=f"psc{m}{hf}{i}") for i in range(4)]
                        first, last = (0, 0, 0), (3, 2, 2)
                        for ci in range(4):
                            for dy in range(3):
                                for dx in range(3):
                                    lhsT = wc[:, ci, (dy * 3 + dx) * 2 + m, :]
                                    for i4 in range(4):
                                        n8 = hf * 4 + i4
                                        rhs = fp[ci][:, n8 * 8 + dy:n8 * 8 + dy + 8,
                                                     dx:dx + 64]
                                        nc.tensor.matmul(
                                            pst[i4], lhsT, rhs,
                                            start=((ci, dy, dx) == first),
                                            stop=((ci, dy, dx) == last))
                        for i4 in range(4):
                            n8 = hf * 4 + i4
                            ysb = sml.tile([128, 512], F32, tag="y")
                            nc.scalar.activation(ysb, pst[i4], AF.Relu,
                                                 bias=bnb[:, m:m + 1],
                                                 scale=bns[:, m:m + 1])
                            nc.sync.dma_start(
                                y_d[m * 128:(m + 1) * 128,
                                    n8 * 512:(n8 + 1) * 512], ysb)
    nc.compile()
    return nc


_PROGRAM = None


def _get_program():
    global _PROGRAM
    if _PROGRAM is None:
        _PROGRAM = _build_program()
    return _PROGRAM


def kernel(F1, F2, Wq, bq, Wk1, bk1, Wv1, bv1, Wk2, bk2, Wv2, bv2,
           mu, Wc, gamma, beta, rmean, rvar):
    f32 = np.float32
    F1 = np.asarray(F1, f32)
    F2 = np.asarray(F2, f32)

    def tile_T(w):   # [O, Cin] -> [128, Cin//128, O] (lhsT tiles)
        wt = np.ascontiguousarray(np.asarray(w, f32).T)      # [Cin, O]
        cin, o = wt.shape
        return wt.reshape(cin // 128, 128, o).transpose(1, 0, 2).astype(_bf)

    wq_h = np.ascontiguousarray(tile_T(Wq))                  # [128, 4, 256]
    wk_h = np.ascontiguousarray(
        np.stack([tile_T(Wk1), tile_T(Wk2)], axis=1))        # [128,2,2,256]
    wv_h = np.ascontiguousarray(
        np.stack([tile_T(Wv1), tile_T(Wv2)], axis=1))

    Wc = np.asarray(Wc, f32)                                 # [256, 512, 3, 3]
    # wc[p, ci, (dy*3+dx)*2+m, col] = Wc[m*128+col, ci*128+p, dy, dx]
    wc_h = Wc.reshape(2, 128, 4, 128, 3, 3)                  # m,col,ci,p,dy,dx
    wc_h = wc_h.transpose(3, 2, 4, 5, 0, 1)                  # p,ci,dy,dx,m,col
    wc_h = np.ascontiguousarray(
        wc_h.reshape(128, 4, 18, 128)).astype(_bf)

    bq_h = np.asarray(bq, f32).reshape(1, 256).astype(_bf)
    # bv_h[p, br, m] = bv_br[m*128 + p]
    bv_h = np.ascontiguousarray(
        np.stack([np.asarray(bv1, f32), np.asarray(bv2, f32)],
                 axis=0).reshape(2, 2, 128).transpose(2, 0, 1))
    inv = np.asarray(gamma, f32) / np.sqrt(np.asarray(rvar, f32) + BN_EPS)
    b2 = np.asarray(beta, f32) - np.asarray(rmean, f32) * inv
    bns_h = np.ascontiguousarray(inv.reshape(2, 128).T)      # [128, 2]
    bnb_h = np.ascontiguousarray(b2.reshape(2, 128).T)
    muv_h = np.full((128, 1), np.asarray(mu, f32).reshape(-1)[0], f32)

    shared = dict(wq=wq_h, wk=wk_h, wv=wv_h, wc=wc_h, bq=bq_h, bv=bv_h,
                  bns=bns_h, bnb=bnb_h, muv=muv_h)
    in_maps = [dict(f1=np.ascontiguousarray(F1[b].reshape(C, HW)).astype(_bf),
                    f2=np.ascontiguousarray(F2[b].reshape(C, HW)).astype(_bf),
                    **shared) for b in range(N_CORES)]

    nc = _get_program()
    res = run_bass_kernel_spmd(nc, in_maps, list(range(N_CORES)))
    kernel.last_results = res

    out = np.stack([res.results[b]["y"] for b in range(N_CORES)])
    return out.reshape(B, C, H, W)


kernel.last_results = None
